# revision 1
# baseline (speedup 1.0000x reference)
"""Trainium2 Bass kernel for BondEncoding2D (Graphormer-style bond encoding).

Computes, for a 512x512 node-pair grid:
  phi_spd[h,i,j]  = spd_table[spatial_pos[i,j], h]
  phi_edge[h,i,j] = (sum_d edge_table[edge_input[i,j,d]] @ W[d])[h] / max(spatial_pos[i,j],1)

Sharding: rows of the grid across 8 NeuronCores (64 rows / 32768 pairs each);
tables and weights replicated (per the sharding hint).

Per-core strategy:
  * Host precomputes M[d] = edge_table @ W[d]; the edge term is then
    edge_sum[pair,:] = sum_d M[d, e_d(pair), :] — 16 gathers of 32-vectors.
  * On device the gather+sum runs on the TensorEngine as a one-hot matmul.
    The 512-wide one-hot over c=(d,bond) is built exactly as
    relu(1 - (e_d - b)^2): a feature matmul (features [1, e_d, e_d^2-split]
    sent from host, exact in bf16) computes the integer argument on the PE,
    and the relu runs natively split across DVE and ACT. The one-hot (exact
    0/1 bf16) is then the stationary operand of matmuls against M (hi/lo
    bf16 split for ~fp32 accuracy), accumulating all 16 hops in PSUM.
    The 1/denom factor is applied in the PSUM->SBUF epilogue (pairs live on
    PSUM partitions, one broadcast multiply per tile).
  * phi_spd is a single exact-f32 GPSIMD ap_gather (runs concurrently with
    the PE pipeline).
  * Host reassembles the (32,512,512) outputs from the device layouts.
"""

import numpy as np
import ml_dtypes

import concourse.bass as bass
import concourse.bacc as bacc
import concourse.mybir as mybir
import concourse.tile as tile
from concourse.bass_utils import run_bass_kernel_spmd

N = 512          # atoms
D = 16           # max_dist
H = 32           # heads
NS = 64          # spatial values
NCORES = 8
RC = N // NCORES          # rows per core (64)
PC = RC * N               # pairs per core (32768)

TILES = 64                # PE tiles per core
TP = 512                  # pairs per tile (= one grid row)
NG = 4                    # groups of 128 pairs per tile
STAGE_T = 8               # tiles batched per output DMA
NF = 64                   # feature rows (1 + 16*3 used, padded)

# spd gather side
NBLK = 8                  # pair blocks (one per Q7 core)
SCALLS = 8                # gather calls
SJT = PC // NBLK // SCALLS  # pairs per block per call (512)

BF16 = mybir.dt.bfloat16
F32 = mybir.dt.float32
I16 = mybir.dt.int16

_cached = {}


def _build_nc(bench_reps=None, parts=("spd", "edge"), INTERLEAVE=True, EPI="alt"):
    nc = bacc.Bacc(None, target_bir_lowering=False)

    afeat = nc.dram_tensor("afeat", [128, 512], BF16, kind="ExternalInput")
    mflat = nc.dram_tensor("mflat", [128, 256], BF16, kind="ExternalInput")
    feat = nc.dram_tensor("feat", [NF, PC], BF16, kind="ExternalInput")
    tab_s = nc.dram_tensor("tab_s", [128, 4096 * 4], F32, kind="ExternalInput")
    sidx = nc.dram_tensor("sidx", [128, SCALLS * SJT // 32], I16,
                          kind="ExternalInput")
    rdev = nc.dram_tensor("rdev", [128, PC // 128], F32, kind="ExternalInput")
    oedge = nc.dram_tensor("oedge", [128, PC // 128 * H], F32,
                           kind="ExternalOutput")
    ospd = nc.dram_tensor("ospd", [128, SCALLS * SJT * 2], F32,
                          kind="ExternalOutput")

    mult = mybir.AluOpType.mult
    RELU = mybir.ActivationFunctionType.Relu

    with tile.TileContext(nc) as tc:
        with (
            tc.tile_pool(name="consts", bufs=1) as cpool,
            tc.tile_pool(name="arg_a", bufs=2, space="PSUM") as agpool_a,
            tc.tile_pool(name="arg_b", bufs=1, space="PSUM") as agpool_b,
            tc.tile_pool(name="outp", bufs=2, space="PSUM") as oppool,
            tc.tile_pool(name="ct", bufs=3) as ctpool,
            tc.tile_pool(name="stage", bufs=2) as stpool,
            tc.tile_pool(name="spd", bufs=2) as spool,
        ):
            afeat_t = cpool.tile([128, 512], BF16)
            nc.sync.dma_start(afeat_t[:], afeat[:])
            mflat_t = cpool.tile([128, 256], BF16)
            nc.sync.dma_start(mflat_t[:], mflat[:])
            tabs_t = cpool.tile([128, 4096 * 4], F32)
            nc.sync.dma_start(tabs_t[:], tab_s[:])
            rdev_t = cpool.tile([128, PC // 128], F32)
            nc.sync.dma_start(rdev_t[:], rdev[:])
            featrep = cpool.tile([128, PC], BF16)
            for half in range(2):
                nc.sync.dma_start(featrep[64 * half:64 * half + NF, :], feat[:])

            import contextlib
            loop_cm = (
                tc.For_i(0, bench_reps, 1) if bench_reps
                else contextlib.nullcontext()
            )
            with loop_cm:
                # ---- phi_edge PE pipeline with spd gathers interleaved
                def spd_call(s):
                    si = spool.tile([128, SJT // 32], I16, tag="si")
                    nc.sync.dma_start(
                        si[:], sidx[:, s * (SJT // 32):(s + 1) * (SJT // 32)]
                    )
                    gs = spool.tile([128, SJT * 2], F32, tag="gs")
                    nc.gpsimd.ap_gather(
                        gs[:].rearrange("p (i v) -> p i v", v=4),
                        tabs_t[:].rearrange("p (n v) -> p n v", v=4),
                        si[:], channels=128, num_elems=4096, d=4,
                        num_idxs=SJT // 2,
                    )
                    nc.sync.dma_start(
                        ospd[:, s * SJT * 2:(s + 1) * SJT * 2], gs[:]
                    )

                if "edge" not in parts or not INTERLEAVE:
                    for s in range(SCALLS if "spd" in parts else 0):
                        spd_call(s)
                import collections
                pend = collections.deque()
                SKEW = 2
                ntl = TILES if "edge" in parts else 0
                for tt in range(ntl + SKEW):
                    if tt < ntl:
                        t = tt
                        if ("spd" in parts and INTERLEAVE
                                and t % (TILES // SCALLS) == 0):
                            spd_call(t // (TILES // SCALLS))
                        # one-hot args: arg[c,pair] = 1-(e_d(c)-b(c))^2 on PE
                        args = []
                        for ab in range(2):   # two double-bank psum tiles
                            pool = agpool_a if ab == 0 else agpool_b
                            ag = pool.tile([128, 2 * TP], F32, tag=f"ag{ab}")
                            for qq in range(2):   # chunks q = 2*ab + qq
                                q = 2 * ab + qq
                                nc.tensor.matmul(
                                    ag[:, qq * TP:(qq + 1) * TP],
                                    afeat_t[64 * qq:64 * qq + 64,
                                            128 * q:128 * q + 128],
                                    featrep[64 * qq:64 * qq + 64,
                                            t * TP:(t + 1) * TP],
                                    start=True, stop=True,
                                    tile_position=(64 * qq, 0),
                                )
                            args.append(ag)
                        # relu -> exact one-hot bf16 (DVE half, ACT half)
                        ctA = ctpool.tile([128, 2 * TP], BF16, tag="ctA")
                        nc.vector.tensor_relu(ctA[:], args[0][:])
                        ctB = ctpool.tile([128, 2 * TP], BF16, tag="ctB")
                        nc.scalar.activation(ctB[:], args[1][:], RELU)
                        pend.append((t, ctA, ctB))
                    if tt < SKEW - 1 or not pend:
                        continue
                    if tt < ntl and len(pend) <= SKEW - 1:
                        continue
                    t, ctA, ctB = pend.popleft()
                    # main matmuls: accumulate 4 chunks x (hi,lo) per group
                    op = oppool.tile([128, 128], F32, tag="op")
                    for gg in range(NG):
                        for q in range(4):
                            ct = ctA if q < 2 else ctB
                            qq = q % 2
                            for h in range(2):
                                nc.tensor.matmul(
                                    op[:, 32 * gg:32 * gg + 32],
                                    ct[:, qq * TP + 128 * gg:
                                       qq * TP + 128 * gg + 128],
                                    mflat_t[:, 64 * q + 32 * h:
                                            64 * q + 32 * h + 32],
                                    start=(q == 0 and h == 0),
                                    stop=(q == 3 and h == 1),
                                )
                    # epilogue: x (1/denom) broadcast, PSUM -> staging SBUF
                    sl = t % STAGE_T
                    if sl == 0:
                        st = stpool.tile([128, 128 * STAGE_T], F32, tag="st")
                    r4 = rdev_t[:, t * NG:(t + 1) * NG]
                    if EPI == "act":
                        COPY = mybir.ActivationFunctionType.Copy
                        for gg in range(NG):
                            nc.scalar.activation(
                                st[:, 128 * sl + 32 * gg:
                                   128 * sl + 32 * gg + 32],
                                op[:, 32 * gg:32 * gg + 32],
                                COPY, scale=r4[:, gg:gg + 1],
                            )
                    else:
                        r4b = r4.rearrange("p (g o) -> p g o", o=1)
                        r4b = r4b.broadcast_to((128, NG, H))
                        stv = st[:, 128 * sl:128 * (sl + 1)].rearrange(
                            "p (g k) -> p g k", g=NG)
                        opv = op[:].rearrange("p (g k) -> p g k", g=NG)
                        eng = (nc.vector if (EPI == "dve" or t % 2 == 0)
                               else nc.any)
                        eng.tensor_tensor(stv, opv, r4b, mult)
                    if sl == STAGE_T - 1:
                        b = t // STAGE_T
                        nc.sync.dma_start(
                            oedge[:, b * 128 * STAGE_T:(b + 1) * 128 * STAGE_T],
                            st[:],
                        )
    nc.compile()
    return nc


def _host_prep(spatial_pos, edge_input, max_dist, spd_table, edge_table,
               edge_dis_weight):
    """Build per-core input maps (all numpy)."""
    md = int(max_dist)
    assert md == D
    W = edge_dis_weight.reshape(-1, H, H)[:md].astype(np.float64)
    M = edge_table.astype(np.float64) @ W          # (16, 32, 32)

    cp = np.arange(128)
    bb = (cp % 32).astype(np.float64)              # bond id per c'
    # feature-matmul weights: arg = (1-b^2)*1 + 2b*e_d - e2hi_d - e2lo_d
    afeat = np.zeros((128, 512), np.float64)
    for q in range(4):
        dsel = 4 * q + cp // 32                    # d(c') per column
        blk = np.zeros((64, 128))
        cc = 1.0 - bb * bb
        cchi = cc.astype(ml_dtypes.bfloat16).astype(np.float64)
        blk[0, :] = cchi
        blk[49, :] = cc - cchi
        blk[1 + dsel, cp] = 2.0 * bb
        blk[17 + dsel, cp] = -1.0
        blk[33 + dsel, cp] = -1.0
        afeat[64 * (q % 2):64 * (q % 2) + 64, 128 * q:128 * q + 128] = blk
    afeat = afeat.astype(ml_dtypes.bfloat16)

    # mflat[c', 64q+32h+k] = hi/lo bf16 split of M[4q + c'//32, c'%32, k]
    mflat = np.zeros((128, 256), ml_dtypes.bfloat16)
    for q in range(4):
        blk = M[4 * q + cp // 32, cp % 32, :]      # (128, 32) float64
        hi = blk.astype(ml_dtypes.bfloat16)
        lo = (blk - hi.astype(np.float64)).astype(ml_dtypes.bfloat16)
        mflat[:, 64 * q:64 * q + 32] = hi
        mflat[:, 64 * q + 32:64 * q + 64] = lo

    # spd pair-combined gather table: partition 16g+hh holds head cols
    # (2hh, 2hh+1) for both members of the pair-pair (sa, sb) = (c//64, c%64)
    hh = (np.arange(128) % 16)
    X = np.ascontiguousarray(
        spd_table[:, np.stack([2 * hh, 2 * hh + 1], 1)].transpose(1, 0, 2)
    ).astype(np.float32)                           # (128, 64, 2)
    T4 = np.empty((128, NS, NS, 4), np.float32)
    T4[:, :, :, 0:2] = X[:, :, None, :]
    T4[:, :, :, 2:4] = X[:, None, :, :]
    tab_s = T4.reshape(128, 4096 * 4)

    in_maps = []
    for c in range(NCORES):
        rows = slice(RC * c, RC * (c + 1))
        e = edge_input[rows].reshape(PC, D).astype(np.float64)
        e2 = e * e
        e2hi = e2.astype(ml_dtypes.bfloat16)
        e2lo = (e2 - e2hi.astype(np.float64)).astype(ml_dtypes.bfloat16)
        feat = np.zeros((NF, PC), ml_dtypes.bfloat16)
        feat[0, :] = 1.0
        feat[49, :] = 1.0
        feat[1:17, :] = e.T.astype(ml_dtypes.bfloat16)
        feat[17:33, :] = e2hi.T
        feat[33:49, :] = e2lo.T
        sp = spatial_pos[rows].reshape(PC).astype(np.int32)
        sp2 = NS * sp[0::2] + sp[1::2]             # combined pair-pair idx
        sw = sp2.reshape(NBLK, SCALLS, SJT // 32, 16).transpose(0, 3, 1, 2)
        sidx = np.ascontiguousarray(sw).reshape(128, SCALLS * SJT // 32)
        rdev = (1.0 / np.maximum(sp, 1)).astype(np.float32)
        rdev = np.ascontiguousarray(rdev.reshape(PC // 128, 128).T)
        in_maps.append({
            "afeat": afeat, "mflat": mflat, "feat": feat,
            "tab_s": tab_s, "sidx": sidx.astype(np.int16), "rdev": rdev,
        })
    return in_maps


def _host_assemble(results):
    phi_spd = np.empty((H, N, N), np.float32)
    phi_edge = np.empty((H, N, N), np.float32)
    for c in range(NCORES):
        rs = slice(RC * c, RC * (c + 1))
        a = results[c]["ospd"].reshape(NBLK, 16, SCALLS, SJT // 2, 2, 2)
        phi_spd[:, rs, :] = a.transpose(1, 5, 0, 2, 3, 4).reshape(H, RC, N)
        b = results[c]["oedge"].reshape(128, TILES, NG, H)
        phi_edge[:, rs, :] = b.transpose(3, 1, 2, 0).reshape(H, RC, N)
    return phi_spd, phi_edge


def kernel(spatial_pos, edge_input, max_dist, spd_table, edge_table,
           edge_dis_weight, _trace=False):
    spatial_pos = np.asarray(spatial_pos)
    edge_input = np.asarray(edge_input)
    spd_table = np.asarray(spd_table, dtype=np.float32)
    edge_table = np.asarray(edge_table, dtype=np.float32)
    edge_dis_weight = np.asarray(edge_dis_weight, dtype=np.float32)

    if "nc" not in _cached:
        _cached["nc"] = _build_nc()
    nc = _cached["nc"]

    in_maps = _host_prep(spatial_pos, edge_input, max_dist, spd_table,
                         edge_table, edge_dis_weight)
    res = run_bass_kernel_spmd(
        nc, in_maps, core_ids=list(range(NCORES)), trace=bool(_trace)
    )
    out = _host_assemble(res.results)
    if _trace:
        return out, res
    return out

